# revision 9
# baseline (speedup 1.0000x reference)
"""Trainium2 Bass kernel for ByteMemory: FNV 3-gram hash + embedding gather.

Full inputs: input_bytes [32, 8192] int32, memory_table [1_000_000, 128] f32.
Full output: [32, 8190, 128] f32 = memory_table[fnv_hash(input_bytes) % 1e6].

Sharding: data parallel over the batch — core k handles rows 4k..4k+3 and
receives a replicated memory_table. Each core's 4x8192 bytes are pre-chunked
on the host into a [128, 258] tile (partition p = row*32 + chunk holds bytes
[chunk*256, chunk*256+258) of its row, zero-padded past the row end), so every
partition computes 256 sliding-window hashes on the DVE and the table rows are
fetched with chunked indirect DMAs (SWDGE gather, one index per partition per
instruction — the only form the HW ucode supports; see the note in _build_nc).
The gather stream on the Pool engine (~1.4us per 128 rows) is the critical
path; hashing (DVE), byte load and output writes (HWDGE) all pipeline under
it, and a small first group minimizes the serial hash lead-in.

The FNV multiply (mod 2^32) and mod-1e6 are decomposed into 16/8-bit limbs:
the DVE ALU is fp32 internally, so every product/sum is kept below 2^24 where
fp32 integer arithmetic is exact; bit splits use bitwise ops (bit-exact).
"""
import numpy as np

import concourse.bacc as bacc
import concourse.bass as bass
import concourse.bass_interp as bass_interp  # noqa: F401 (import keeps parity with sim use)
import concourse.mybir as mybir
import concourse.tile as tile
from concourse.bass_utils import run_bass_kernel_spmd

OP = mybir.AluOpType

# ---- problem constants (hardcoded per harness contract) ----
B, L = 32, 8192
NGRAM = 3
OUT_LEN = L - NGRAM + 1  # 8190
CAPACITY = 1_000_000
D = 128
N_CORES = 8
ROWS_PER_CORE = B // N_CORES  # 4
CHUNKS_PER_ROW = 32
SEG = 256  # windows per partition
SEGB = SEG + 2  # bytes needed per partition
P = 128  # partitions

# windows per gather group (one hash batch + one output write per group).
# The first group is small so its hash (the serial lead-in before the Pool
# engine's gather stream starts) finishes quickly; the last group is small so
# the final output write (which drains after the last gather) is short.
GROUPS = [16, 48, 64, 64, 56, 8]
assert sum(GROUPS) == SEG

SEED = 0x12345678
FNV = 16777619  # 2^24 + 403

_K1 = (SEED * FNV) & 0xFFFFFFFF
_K1_LO8 = _K1 & 0xFF
_K1_HI24 = _K1 & 0xFFFFFF00
_K2 = (_K1_HI24 * FNV) & 0xFFFFFFFF
_K2_LO = _K2 & 0xFFFF
_K2_HI = _K2 >> 16


def _build_hash_index(nc, pool, bytes_tile, idx_out, n, col0=0, tag=""):
    """Emit DVE ops computing idx_out[:, 0:n] (FNV3 % 1e6) from
    bytes_tile[:, col0:col0+n+2]. idx_out must be a contiguous [128, n] tile
    (the HW indirect-DMA offset AP requires a zero-offset contiguous tile)."""
    dt = mybir.dt

    def t32(name):
        return pool.tile([P, n], dt.int32, tag=f"h{tag}_{name}", name=f"h{tag}_{name}")

    def tf(name):
        return pool.tile([P, n], dt.float32, tag=f"h{tag}_{name}", name=f"h{tag}_{name}")

    b0 = bytes_tile[:, col0 : col0 + n]
    b1 = bytes_tile[:, col0 + 1 : col0 + n + 1]
    b2 = bytes_tile[:, col0 + 2 : col0 + n + 2]
    out = idx_out[:, 0:n]

    V = nc.vector

    # round 2: h2 = (h1 * FNV) ^ b1, with h1 = K1 ^ b0 = K1_HI24 + v
    v = t32("v")
    V.tensor_scalar(out=v[:], in0=b0, scalar1=_K1_LO8, scalar2=None, op0=OP.bitwise_xor)
    mt = t32("mt")
    V.tensor_scalar(out=mt[:], in0=v[:], scalar1=403, scalar2=_K2_LO, op0=OP.mult, op1=OP.add)
    lo2t = t32("lo2t")
    V.tensor_scalar(out=lo2t[:], in0=mt[:], scalar1=0xFFFF, scalar2=None, op0=OP.bitwise_and)
    cr2 = t32("cr2")
    V.tensor_scalar(out=cr2[:], in0=mt[:], scalar1=16, scalar2=None, op0=OP.logical_shift_right)
    u = t32("u")
    V.tensor_scalar(out=u[:], in0=v[:], scalar1=256, scalar2=_K2_HI, op0=OP.mult, op1=OP.add)
    u2 = t32("u2")
    V.tensor_tensor(out=u2[:], in0=u[:], in1=cr2[:], op=OP.add)
    hi2 = t32("hi2")
    V.tensor_scalar(out=hi2[:], in0=u2[:], scalar1=0xFFFF, scalar2=None, op0=OP.bitwise_and)
    lo2 = t32("lo2")
    V.tensor_tensor(out=lo2[:], in0=lo2t[:], in1=b1, op=OP.bitwise_xor)

    # round 3: h3 = (h2 * FNV) ^ b2, h2 = hi2*2^16 + lo2
    lo_l = t32("lo_l")
    V.tensor_scalar(out=lo_l[:], in0=lo2[:], scalar1=0xFF, scalar2=None, op0=OP.bitwise_and)
    lo_h = t32("lo_h")
    V.tensor_scalar(out=lo_h[:], in0=lo2[:], scalar1=8, scalar2=None, op0=OP.logical_shift_right)
    A = t32("A")
    V.tensor_scalar(out=A[:], in0=lo_l[:], scalar1=403, scalar2=None, op0=OP.mult)
    Bt = t32("Bt")
    V.tensor_scalar(out=Bt[:], in0=lo_h[:], scalar1=403, scalar2=None, op0=OP.mult)
    Bl8 = t32("Bl8")
    V.tensor_scalar(out=Bl8[:], in0=Bt[:], scalar1=0xFF, scalar2=8, op0=OP.bitwise_and, op1=OP.logical_shift_left)
    mlo = t32("mlo")
    V.tensor_tensor(out=mlo[:], in0=A[:], in1=Bl8[:], op=OP.add)
    lo3t = t32("lo3t")
    V.tensor_scalar(out=lo3t[:], in0=mlo[:], scalar1=0xFFFF, scalar2=None, op0=OP.bitwise_and)
    cr3 = t32("cr3")
    V.tensor_scalar(out=cr3[:], in0=mlo[:], scalar1=16, scalar2=None, op0=OP.logical_shift_right)
    Bh = t32("Bh")
    V.tensor_scalar(out=Bh[:], in0=Bt[:], scalar1=8, scalar2=None, op0=OP.logical_shift_right)
    hi_l = t32("hi_l")
    V.tensor_scalar(out=hi_l[:], in0=hi2[:], scalar1=0xFF, scalar2=None, op0=OP.bitwise_and)
    hi_h = t32("hi_h")
    V.tensor_scalar(out=hi_h[:], in0=hi2[:], scalar1=8, scalar2=None, op0=OP.logical_shift_right)
    Dm = t32("Dm")
    V.tensor_scalar(out=Dm[:], in0=hi_l[:], scalar1=403, scalar2=None, op0=OP.mult)
    E = t32("E")
    V.tensor_scalar(out=E[:], in0=hi_h[:], scalar1=403, scalar2=None, op0=OP.mult)
    El = t32("El")
    V.tensor_scalar(out=El[:], in0=E[:], scalar1=0xFF, scalar2=None, op0=OP.bitwise_and)
    hc = t32("hc")
    V.scalar_tensor_tensor(out=hc[:], in0=El[:], scalar=256, in1=Dm[:], op0=OP.mult, op1=OP.add)
    lol8 = t32("lol8")
    V.tensor_scalar(out=lol8[:], in0=lo_l[:], scalar1=256, scalar2=None, op0=OP.mult)
    S1 = t32("S1")
    V.tensor_tensor(out=S1[:], in0=Bh[:], in1=cr3[:], op=OP.add)
    S2 = t32("S2")
    V.tensor_tensor(out=S2[:], in0=S1[:], in1=hc[:], op=OP.add)
    S3 = t32("S3")
    V.tensor_tensor(out=S3[:], in0=S2[:], in1=lol8[:], op=OP.add)
    hi3 = t32("hi3")
    V.tensor_scalar(out=hi3[:], in0=S3[:], scalar1=0xFFFF, scalar2=None, op0=OP.bitwise_and)
    lo3 = t32("lo3")
    V.tensor_tensor(out=lo3[:], in0=lo3t[:], in1=b2, op=OP.bitwise_xor)

    # mod 1e6: idx = (hi3*2^16 + lo3) mod 1e6
    hf = tf("hf")
    V.tensor_scalar(out=hf[:], in0=hi3[:], scalar1=65536.0, scalar2=None, op0=OP.mult)
    hf2 = tf("hf2")
    V.tensor_tensor(out=hf2[:], in0=hf[:], in1=lo3[:], op=OP.add)
    qf = tf("qf")
    V.tensor_scalar(out=qf[:], in0=hf2[:], scalar1=1.0 / 1.0e6, scalar2=None, op0=OP.mult)
    q = t32("q")
    V.tensor_copy(out=q[:], in_=qf[:])
    qm = t32("qm")
    V.tensor_scalar(out=qm[:], in0=q[:], scalar1=244, scalar2=None, op0=OP.mult)
    u12 = t32("u12")
    V.tensor_scalar(out=u12[:], in0=qm[:], scalar1=0xFFF, scalar2=None, op0=OP.bitwise_and)
    w = t32("w")
    V.tensor_scalar(out=w[:], in0=q[:], scalar1=576, scalar2=None, op0=OP.mult)
    wh = t32("wh")
    V.tensor_scalar(out=wh[:], in0=w[:], scalar1=12, scalar2=None, op0=OP.logical_shift_right)
    wl = t32("wl")
    V.tensor_scalar(out=wl[:], in0=w[:], scalar1=0xFFF, scalar2=None, op0=OP.bitwise_and)
    s = t32("s")
    V.tensor_tensor(out=s[:], in0=u12[:], in1=wh[:], op=OP.add)
    v2 = t32("v2")
    V.tensor_scalar(out=v2[:], in0=s[:], scalar1=0xFFF, scalar2=12, op0=OP.bitwise_and, op1=OP.logical_shift_left)
    y = t32("y")
    V.tensor_tensor(out=y[:], in0=v2[:], in1=wl[:], op=OP.add)
    hmt = t32("hmt")
    V.tensor_scalar(out=hmt[:], in0=hi3[:], scalar1=0xFF, scalar2=16, op0=OP.bitwise_and, op1=OP.logical_shift_left)
    hm = t32("hm")
    V.tensor_tensor(out=hm[:], in0=hmt[:], in1=lo3[:], op=OP.add)
    r24 = t32("r24")
    V.tensor_tensor(out=r24[:], in0=hm[:], in1=y[:], op=OP.subtract)
    m1 = t32("m1")
    V.tensor_scalar(out=m1[:], in0=r24[:], scalar1=float(2**23), scalar2=float(2**24), op0=OP.is_ge, op1=OP.mult)
    ra = t32("ra")
    V.tensor_tensor(out=ra[:], in0=r24[:], in1=m1[:], op=OP.subtract)
    m2 = t32("m2")
    V.tensor_scalar(out=m2[:], in0=ra[:], scalar1=float(-(2**23)), scalar2=float(2**24), op0=OP.is_lt, op1=OP.mult)
    rb = t32("rb")
    V.tensor_tensor(out=rb[:], in0=ra[:], in1=m2[:], op=OP.add)
    cur = rb
    for i, (thr, opc, sign) in enumerate(
        [(0.0, OP.is_lt, OP.add), (1.0e6, OP.is_ge, OP.subtract)]
    ):
        msk = t32(f"msk{i}")
        V.tensor_scalar(out=msk[:], in0=cur[:], scalar1=thr, scalar2=1.0e6, op0=opc, op1=OP.mult)
        if i < 1:
            nxt = t32(f"fix{i}")
            V.tensor_tensor(out=nxt[:], in0=cur[:], in1=msk[:], op=sign)
            cur = nxt
        else:
            V.tensor_tensor(out=out, in0=cur[:], in1=msk[:], op=sign)


def _build_nc():
    nc = bacc.Bacc("TRN2", target_bir_lowering=False, debug=False)
    tbl_d = nc.dram_tensor("memory_table", [CAPACITY, D], mybir.dt.float32, kind="ExternalInput").ap()
    byt_d = nc.dram_tensor("bytes_chunks", [P, SEGB], mybir.dt.int32, kind="ExternalInput").ap()
    out_d = nc.dram_tensor("out", [P, SEG * D], mybir.dt.float32, kind="ExternalOutput").ap()

    with tile.TileContext(nc) as tc:
        with tc.tile_pool(name="hash", bufs=2) as hpool, \
             tc.tile_pool(name="const", bufs=1) as cpool, \
             tc.tile_pool(name="idx", bufs=4) as ipool, \
             tc.tile_pool(name="gather", bufs=4) as gpool:
            bt = cpool.tile([P, SEGB], mybir.dt.int32, tag="bt", name="bt")
            nc.sync.dma_start(out=bt[:], in_=byt_d[:])

            c0 = 0
            for g, gw in enumerate(GROUPS):
                it = ipool.tile([P, gw], mybir.dt.int32, tag="it", name=f"it{g}")
                _build_hash_index(nc, hpool, bt, it, gw, col0=c0)
                gt = gpool.tile([P, gw * D], mybir.dt.float32, tag="gt", name=f"gt{g}")
                # the HW indirect-DMA ucode honors exactly one index per
                # partition per instruction (multi-index offset APs silently
                # read contiguously from the first index, and strided multi-run
                # dests scramble), so issue gw single-index gathers per group;
                # the hash (DVE) and output writes (HWDGE/sync) pipeline under
                # the Pool-engine gather stream, which is the critical path at
                # ~1.4us per 128 gathered rows.
                for j in range(gw):
                    nc.gpsimd.indirect_dma_start(
                        out=gt[:, j * D : (j + 1) * D],
                        out_offset=None,
                        in_=tbl_d[:],
                        in_offset=bass.IndirectOffsetOnAxis(ap=it[:, j : j + 1], axis=0),
                    )
                nc.sync.dma_start(out=out_d[:, c0 * D : (c0 + gw) * D], in_=gt[:])
                c0 += gw

    nc.compile()
    return nc


_NC_CACHE = {}


def _get_nc():
    if "nc" not in _NC_CACHE:
        _NC_CACHE["nc"] = _build_nc()
    return _NC_CACHE["nc"]


def _chunk_bytes(rows: np.ndarray) -> np.ndarray:
    """rows [ROWS_PER_CORE, L] int32 -> [128, SEGB] int32 overlapping windows."""
    out = np.zeros((P, SEGB), dtype=np.int32)
    for r in range(ROWS_PER_CORE):
        for c in range(CHUNKS_PER_ROW):
            seg = rows[r, c * SEG : min(c * SEG + SEGB, L)]
            out[r * CHUNKS_PER_ROW + c, : len(seg)] = seg
    return out


def kernel(input_bytes: np.ndarray, memory_table: np.ndarray, **_kw) -> np.ndarray:
    input_bytes = np.ascontiguousarray(np.asarray(input_bytes, dtype=np.int32))
    memory_table = np.ascontiguousarray(np.asarray(memory_table, dtype=np.float32))
    assert input_bytes.shape == (B, L)
    assert memory_table.shape == (CAPACITY, D)

    nc = _get_nc()
    in_maps = []
    for k in range(N_CORES):
        rows = input_bytes[k * ROWS_PER_CORE : (k + 1) * ROWS_PER_CORE]
        in_maps.append({
            "memory_table": memory_table,
            "bytes_chunks": _chunk_bytes(rows),
        })
    res = run_bass_kernel_spmd(nc, in_maps, core_ids=list(range(N_CORES)))
    parts = [
        res.results[k]["out"].reshape(ROWS_PER_CORE, L, D)[:, :OUT_LEN, :]
        for k in range(N_CORES)
    ]
    return np.concatenate(parts, axis=0)


# revision 10
# speedup vs baseline: 1.1705x; 1.1705x over previous
"""Trainium2 Bass kernel for ByteMemory: FNV 3-gram hash + embedding gather.

Full inputs: input_bytes [32, 8192] int32, memory_table [1_000_000, 128] f32.
Full output: [32, 8190, 128] f32 = memory_table[fnv_hash(input_bytes) % 1e6].

Sharding: data parallel over the batch — core k handles rows 4k..4k+3 and
receives a replicated memory_table. Each core's 4x8192 bytes are pre-chunked
on the host into a [128, 258] tile (partition p = row*32 + chunk holds bytes
[chunk*256, chunk*256+258) of its row, zero-padded past the row end), so every
partition computes 256 sliding-window hashes on the DVE and the table rows are
fetched with chunked indirect DMAs (SWDGE gather, one index per partition per
instruction — the only form the HW ucode supports; see the note in _build_nc).
The gather stream on the Pool engine (~1.4us per 128 rows) is the critical
path; hashing (DVE), byte load and output writes (HWDGE) all pipeline under
it, and a small first group minimizes the serial hash lead-in.

The FNV multiply (mod 2^32) and mod-1e6 are decomposed into 16/8-bit limbs:
the DVE ALU is fp32 internally, so every product/sum is kept below 2^24 where
fp32 integer arithmetic is exact; bit splits use bitwise ops (bit-exact).
"""
import numpy as np

import concourse.bacc as bacc
import concourse.bass as bass
import concourse.bass_interp as bass_interp  # noqa: F401 (import keeps parity with sim use)
import concourse.mybir as mybir
import concourse.tile as tile
from concourse.bass_utils import run_bass_kernel_spmd

OP = mybir.AluOpType

# ---- problem constants (hardcoded per harness contract) ----
B, L = 32, 8192
NGRAM = 3
OUT_LEN = L - NGRAM + 1  # 8190
CAPACITY = 1_000_000
D = 128
N_CORES = 8
ROWS_PER_CORE = B // N_CORES  # 4
CHUNKS_PER_ROW = 32
SEG = 256  # windows per partition
SEGB = SEG + 2  # bytes needed per partition
P = 128  # partitions

# windows per gather group (one hash batch + one output write per group).
# The first group is small so its hash (the serial lead-in before the Pool
# engine's gather stream starts) finishes quickly; the last group is small so
# the final output write (which drains after the last gather) is short.
GROUPS = [16, 48, 64, 64, 48, 16]
assert sum(GROUPS) == SEG

SEED = 0x12345678
FNV = 16777619  # 2^24 + 403

_K1 = (SEED * FNV) & 0xFFFFFFFF
_K1_LO8 = _K1 & 0xFF
_K1_HI24 = _K1 & 0xFFFFFF00
_K2 = (_K1_HI24 * FNV) & 0xFFFFFFFF
_K2_LO = _K2 & 0xFFFF
_K2_HI = _K2 >> 16


def _build_hash_index(nc, pool, bytes_tile, idx_out, n, col0=0, tag=""):
    """Emit DVE ops computing idx_out[:, 0:n] (FNV3 % 1e6) from
    bytes_tile[:, col0:col0+n+2]. idx_out must be a contiguous [128, n] tile
    (the HW indirect-DMA offset AP requires a zero-offset contiguous tile)."""
    dt = mybir.dt

    def t32(name):
        return pool.tile([P, n], dt.int32, tag=f"h{tag}_{name}", name=f"h{tag}_{name}")

    def tf(name):
        return pool.tile([P, n], dt.float32, tag=f"h{tag}_{name}", name=f"h{tag}_{name}")

    b0 = bytes_tile[:, col0 : col0 + n]
    b1 = bytes_tile[:, col0 + 1 : col0 + n + 1]
    b2 = bytes_tile[:, col0 + 2 : col0 + n + 2]
    out = idx_out[:, 0:n]

    V = nc.vector

    # round 2: h2 = (h1 * FNV) ^ b1, with h1 = K1 ^ b0 = K1_HI24 + v
    v = t32("v")
    V.tensor_scalar(out=v[:], in0=b0, scalar1=_K1_LO8, scalar2=None, op0=OP.bitwise_xor)
    mt = t32("mt")
    V.tensor_scalar(out=mt[:], in0=v[:], scalar1=403, scalar2=_K2_LO, op0=OP.mult, op1=OP.add)
    lo2t = t32("lo2t")
    V.tensor_scalar(out=lo2t[:], in0=mt[:], scalar1=0xFFFF, scalar2=None, op0=OP.bitwise_and)
    cr2 = t32("cr2")
    V.tensor_scalar(out=cr2[:], in0=mt[:], scalar1=16, scalar2=None, op0=OP.logical_shift_right)
    u = t32("u")
    V.tensor_scalar(out=u[:], in0=v[:], scalar1=256, scalar2=_K2_HI, op0=OP.mult, op1=OP.add)
    u2 = t32("u2")
    V.tensor_tensor(out=u2[:], in0=u[:], in1=cr2[:], op=OP.add)
    hi2 = t32("hi2")
    V.tensor_scalar(out=hi2[:], in0=u2[:], scalar1=0xFFFF, scalar2=None, op0=OP.bitwise_and)
    lo2 = t32("lo2")
    V.tensor_tensor(out=lo2[:], in0=lo2t[:], in1=b1, op=OP.bitwise_xor)

    # round 3: h3 = (h2 * FNV) ^ b2, h2 = hi2*2^16 + lo2
    lo_l = t32("lo_l")
    V.tensor_scalar(out=lo_l[:], in0=lo2[:], scalar1=0xFF, scalar2=None, op0=OP.bitwise_and)
    lo_h = t32("lo_h")
    V.tensor_scalar(out=lo_h[:], in0=lo2[:], scalar1=8, scalar2=None, op0=OP.logical_shift_right)
    A = t32("A")
    V.tensor_scalar(out=A[:], in0=lo_l[:], scalar1=403, scalar2=None, op0=OP.mult)
    Bt = t32("Bt")
    V.tensor_scalar(out=Bt[:], in0=lo_h[:], scalar1=403, scalar2=None, op0=OP.mult)
    Bl8 = t32("Bl8")
    V.tensor_scalar(out=Bl8[:], in0=Bt[:], scalar1=0xFF, scalar2=8, op0=OP.bitwise_and, op1=OP.logical_shift_left)
    mlo = t32("mlo")
    V.tensor_tensor(out=mlo[:], in0=A[:], in1=Bl8[:], op=OP.add)
    lo3t = t32("lo3t")
    V.tensor_scalar(out=lo3t[:], in0=mlo[:], scalar1=0xFFFF, scalar2=None, op0=OP.bitwise_and)
    cr3 = t32("cr3")
    V.tensor_scalar(out=cr3[:], in0=mlo[:], scalar1=16, scalar2=None, op0=OP.logical_shift_right)
    Bh = t32("Bh")
    V.tensor_scalar(out=Bh[:], in0=Bt[:], scalar1=8, scalar2=None, op0=OP.logical_shift_right)
    hi_l = t32("hi_l")
    V.tensor_scalar(out=hi_l[:], in0=hi2[:], scalar1=0xFF, scalar2=None, op0=OP.bitwise_and)
    hi_h = t32("hi_h")
    V.tensor_scalar(out=hi_h[:], in0=hi2[:], scalar1=8, scalar2=None, op0=OP.logical_shift_right)
    Dm = t32("Dm")
    V.tensor_scalar(out=Dm[:], in0=hi_l[:], scalar1=403, scalar2=None, op0=OP.mult)
    E = t32("E")
    V.tensor_scalar(out=E[:], in0=hi_h[:], scalar1=403, scalar2=None, op0=OP.mult)
    El = t32("El")
    V.tensor_scalar(out=El[:], in0=E[:], scalar1=0xFF, scalar2=None, op0=OP.bitwise_and)
    hc = t32("hc")
    V.scalar_tensor_tensor(out=hc[:], in0=El[:], scalar=256, in1=Dm[:], op0=OP.mult, op1=OP.add)
    lol8 = t32("lol8")
    V.tensor_scalar(out=lol8[:], in0=lo_l[:], scalar1=256, scalar2=None, op0=OP.mult)
    S1 = t32("S1")
    V.tensor_tensor(out=S1[:], in0=Bh[:], in1=cr3[:], op=OP.add)
    S2 = t32("S2")
    V.tensor_tensor(out=S2[:], in0=S1[:], in1=hc[:], op=OP.add)
    S3 = t32("S3")
    V.tensor_tensor(out=S3[:], in0=S2[:], in1=lol8[:], op=OP.add)
    hi3 = t32("hi3")
    V.tensor_scalar(out=hi3[:], in0=S3[:], scalar1=0xFFFF, scalar2=None, op0=OP.bitwise_and)
    lo3 = t32("lo3")
    V.tensor_tensor(out=lo3[:], in0=lo3t[:], in1=b2, op=OP.bitwise_xor)

    # mod 1e6: idx = (hi3*2^16 + lo3) mod 1e6
    hf = tf("hf")
    V.tensor_scalar(out=hf[:], in0=hi3[:], scalar1=65536.0, scalar2=None, op0=OP.mult)
    hf2 = tf("hf2")
    V.tensor_tensor(out=hf2[:], in0=hf[:], in1=lo3[:], op=OP.add)
    qf = tf("qf")
    V.tensor_scalar(out=qf[:], in0=hf2[:], scalar1=1.0 / 1.0e6, scalar2=None, op0=OP.mult)
    q = t32("q")
    V.tensor_copy(out=q[:], in_=qf[:])
    qm = t32("qm")
    V.tensor_scalar(out=qm[:], in0=q[:], scalar1=244, scalar2=None, op0=OP.mult)
    u12 = t32("u12")
    V.tensor_scalar(out=u12[:], in0=qm[:], scalar1=0xFFF, scalar2=None, op0=OP.bitwise_and)
    w = t32("w")
    V.tensor_scalar(out=w[:], in0=q[:], scalar1=576, scalar2=None, op0=OP.mult)
    wh = t32("wh")
    V.tensor_scalar(out=wh[:], in0=w[:], scalar1=12, scalar2=None, op0=OP.logical_shift_right)
    wl = t32("wl")
    V.tensor_scalar(out=wl[:], in0=w[:], scalar1=0xFFF, scalar2=None, op0=OP.bitwise_and)
    s = t32("s")
    V.tensor_tensor(out=s[:], in0=u12[:], in1=wh[:], op=OP.add)
    v2 = t32("v2")
    V.tensor_scalar(out=v2[:], in0=s[:], scalar1=0xFFF, scalar2=12, op0=OP.bitwise_and, op1=OP.logical_shift_left)
    y = t32("y")
    V.tensor_tensor(out=y[:], in0=v2[:], in1=wl[:], op=OP.add)
    hmt = t32("hmt")
    V.tensor_scalar(out=hmt[:], in0=hi3[:], scalar1=0xFF, scalar2=16, op0=OP.bitwise_and, op1=OP.logical_shift_left)
    hm = t32("hm")
    V.tensor_tensor(out=hm[:], in0=hmt[:], in1=lo3[:], op=OP.add)
    r24 = t32("r24")
    V.tensor_tensor(out=r24[:], in0=hm[:], in1=y[:], op=OP.subtract)
    m1 = t32("m1")
    V.tensor_scalar(out=m1[:], in0=r24[:], scalar1=float(2**23), scalar2=float(2**24), op0=OP.is_ge, op1=OP.mult)
    ra = t32("ra")
    V.tensor_tensor(out=ra[:], in0=r24[:], in1=m1[:], op=OP.subtract)
    m2 = t32("m2")
    V.tensor_scalar(out=m2[:], in0=ra[:], scalar1=float(-(2**23)), scalar2=float(2**24), op0=OP.is_lt, op1=OP.mult)
    rb = t32("rb")
    V.tensor_tensor(out=rb[:], in0=ra[:], in1=m2[:], op=OP.add)
    cur = rb
    for i, (thr, opc, sign) in enumerate(
        [(0.0, OP.is_lt, OP.add), (1.0e6, OP.is_ge, OP.subtract)]
    ):
        msk = t32(f"msk{i}")
        V.tensor_scalar(out=msk[:], in0=cur[:], scalar1=thr, scalar2=1.0e6, op0=opc, op1=OP.mult)
        if i < 1:
            nxt = t32(f"fix{i}")
            V.tensor_tensor(out=nxt[:], in0=cur[:], in1=msk[:], op=sign)
            cur = nxt
        else:
            V.tensor_tensor(out=out, in0=cur[:], in1=msk[:], op=sign)


def _build_nc():
    nc = bacc.Bacc("TRN2", target_bir_lowering=False, debug=False)
    tbl_d = nc.dram_tensor("memory_table", [CAPACITY, D], mybir.dt.float32, kind="ExternalInput").ap()
    byt_d = nc.dram_tensor("bytes_chunks", [P, SEGB], mybir.dt.int32, kind="ExternalInput").ap()
    out_d = nc.dram_tensor("out", [P, SEG * D], mybir.dt.float32, kind="ExternalOutput").ap()

    with tile.TileContext(nc) as tc:
        with tc.tile_pool(name="hash", bufs=2) as hpool, \
             tc.tile_pool(name="const", bufs=1) as cpool, \
             tc.tile_pool(name="idx", bufs=4) as ipool, \
             tc.tile_pool(name="gather", bufs=4) as gpool:
            bt = cpool.tile([P, SEGB], mybir.dt.int32, tag="bt", name="bt")
            nc.sync.dma_start(out=bt[:], in_=byt_d[:])

            c0 = 0
            for g, gw in enumerate(GROUPS):
                it = ipool.tile([P, gw], mybir.dt.int32, tag="it", name=f"it{g}")
                _build_hash_index(nc, hpool, bt, it, gw, col0=c0)
                gt = gpool.tile([P, gw * D], mybir.dt.float32, tag="gt", name=f"gt{g}")
                # the HW indirect-DMA ucode honors exactly one index per
                # partition per instruction (multi-index offset APs silently
                # read contiguously from the first index, and strided multi-run
                # dests scramble), so issue gw single-index gathers per group;
                # the hash (DVE) and output writes (HWDGE/sync) pipeline under
                # the Pool-engine gather stream, which is the critical path at
                # ~1.4us per 128 gathered rows.
                for j in range(gw):
                    nc.gpsimd.indirect_dma_start(
                        out=gt[:, j * D : (j + 1) * D],
                        out_offset=None,
                        in_=tbl_d[:],
                        in_offset=bass.IndirectOffsetOnAxis(ap=it[:, j : j + 1], axis=0),
                    )
                nc.sync.dma_start(out=out_d[:, c0 * D : (c0 + gw) * D], in_=gt[:])
                c0 += gw

    nc.compile()
    return nc


_NC_CACHE = {}


def _get_nc():
    if "nc" not in _NC_CACHE:
        _NC_CACHE["nc"] = _build_nc()
    return _NC_CACHE["nc"]


def _chunk_bytes(rows: np.ndarray) -> np.ndarray:
    """rows [ROWS_PER_CORE, L] int32 -> [128, SEGB] int32 overlapping windows."""
    out = np.zeros((P, SEGB), dtype=np.int32)
    for r in range(ROWS_PER_CORE):
        for c in range(CHUNKS_PER_ROW):
            seg = rows[r, c * SEG : min(c * SEG + SEGB, L)]
            out[r * CHUNKS_PER_ROW + c, : len(seg)] = seg
    return out


def kernel(input_bytes: np.ndarray, memory_table: np.ndarray, **_kw) -> np.ndarray:
    input_bytes = np.ascontiguousarray(np.asarray(input_bytes, dtype=np.int32))
    memory_table = np.ascontiguousarray(np.asarray(memory_table, dtype=np.float32))
    assert input_bytes.shape == (B, L)
    assert memory_table.shape == (CAPACITY, D)

    nc = _get_nc()
    in_maps = []
    for k in range(N_CORES):
        rows = input_bytes[k * ROWS_PER_CORE : (k + 1) * ROWS_PER_CORE]
        in_maps.append({
            "memory_table": memory_table,
            "bytes_chunks": _chunk_bytes(rows),
        })
    res = run_bass_kernel_spmd(nc, in_maps, core_ids=list(range(N_CORES)))
    parts = [
        res.results[k]["out"].reshape(ROWS_PER_CORE, L, D)[:, :OUT_LEN, :]
        for k in range(N_CORES)
    ]
    return np.concatenate(parts, axis=0)
